# revision 14
# baseline (speedup 1.0000x reference)
"""Style-modulated Conv1d (StyleGAN-like) Trainium2 kernel.

Full-input contract: kernel(**inputs) takes the unsharded fp32 inputs and
returns the full (B, COUT, T) fp32 output. Internally the work is sharded
over 8 NeuronCores: batch-groups of 4 samples x T-halves (4x2 grid), so each
core processes a [128, T/2] slab at full partition occupancy.

The style modulation is folded on the host: with
  s = lrelu(style @ (fc_w * gain)^T + fc_b)          (B, CIN)
  d = rsqrt(sum_{cin,k} (w * s)^2 + eps)             (B, COUT)
the modulated-demodulated conv is an ordinary conv with per-sample taps
  w_final[b, cout, cin, k] = w[cout, cin, k] * s[b, cin] * d[b, cout]
followed by  y = lrelu(conv + nstr*noise + bias).  The taps (a few KB) are
built in fp32 on the host, cast to fp16, and packed block-diagonally over the
4 samples of each core's batch group, so the whole device program is just
K=3 shifted [128x128]x[128x512] matmuls accumulating in PSUM plus a single
ACT Lrelu epilogue (PSUM -> SBUF fp16) per chunk.

x moves over HBM in fp8_e3m4 (TRN FP8_EXP3, 4 mantissa bits) and is fed to
the PE directly as the fp8 moving operand against fp16 stationary taps (the
PE upconverts both to fp22 internally, so no cast pass is needed). y moves
in fp16. End-to-end rel err of the fp8-in pipeline vs the fp32 reference is
1.57e-2 (validated bit-exact against a host emulation), inside the 2e-2
gate. The kernel is PE-bound: 3 taps x 32768 cols = 98304 PE cycles ~ 41us;
HBM traffic is 4.2 MB in + 8.4 MB out per core (~35us at 358 GB/s).

Schedule notes (from NTFF traces):
- input DMAs ride the Sync-engine HWDGE ring; body output DMAs ride the
  GpSimd SWDGE ring (its ~3.5us exit drain overlaps the tail); the last two
  supertiles' outputs are deferred to the Sync ring so nothing waits on the
  GpSimd drain;
- a few warmup matmuls on an uninitialized scratch tile start the PE HAM
  clock ramp while the first x supertile is in flight;
- the epilogue is one ACT op per 1024-col piece: Lrelu(psum*1 + bias) with
  alpha=0.2, writing fp16 straight to the output staging tile (DVE is idle);
- all SBUF tiles live in ONE tile pool (per-tag ring buffers) so the
  end-of-program pool-release barrier storm collapses to a single release.
"""

import ml_dtypes
import numpy as np

import concourse.bass as bass
import concourse.tile as tile
from concourse import bacc, mybir

F32 = mybir.dt.float32
F16 = mybir.dt.float16
F8E3 = mybir.dt.float8e3  # TRN FP8_EXP3 (e3m4): 4 mantissa bits

B, CIN, COUT, T, WDIM, K = 16, 32, 32, 65536, 512, 3
ALPHA = 0.2
GAIN = float(1.0 / np.sqrt(np.float32(WDIM)))
EPS = 1e-8

N_CORES = 8
BG = 4          # samples per core (batch group)
TSPLIT = 2      # T split factor
T_LOC = T // TSPLIT

CH = 1024       # compute chunk columns (one 2-bank PSUM tile, per-chunk out DMA)
MMN = 512       # matmul free dim (one PSUM bank of fp32)
PS_BUFS = 4     # PSUM rotation depth (4 x 2 banks = all 8 banks)
N_WARM = 5      # warmup matmuls to start the HAM clock ramp
N_TAIL_ST = 3   # trailing supertiles with 512-col pieces (short serial tail)


def _supertile_schedule(t_loc):
    """Input-DMA granularity: small head tiles (fast first matmul), 4k body,
    small tail (short drain)."""
    head = [512, 1024, 2048]
    tail = [2048, 512, 512, 512]
    body = t_loc - sum(head) - sum(tail)
    assert body >= 0 and body % 1024 == 0
    widths = head + [4096] * (body // 4096)
    if body % 4096:
        widths.append(body % 4096)
    widths += tail
    assert sum(widths) == t_loc
    return widths


def build_program(t_loc=T_LOC, with_noise=False, with_bias=False,
                  use_act_lrelu=True):
    """One-core Bass program; identical on all 8 cores (SPMD, data differs)."""
    widths = _supertile_schedule(t_loc)
    mult = mybir.AluOpType.mult
    amax = mybir.AluOpType.max

    nc = bacc.Bacc("TRN2", target_bir_lowering=False, debug=False)
    xh = nc.dram_tensor("xh", [128, t_loc + 2], F8E3, kind="ExternalInput")
    wtk = nc.dram_tensor("wtk", [128, K * 128], F16, kind="ExternalInput")
    if with_bias:
        bia = nc.dram_tensor("bia", [128, 1], F32, kind="ExternalInput")
    if with_noise:
        nz = nc.dram_tensor("nz", [BG, t_loc], F16, kind="ExternalInput")
        wnd = nc.dram_tensor("wnd", [BG, 128], F16, kind="ExternalInput")
    yh = nc.dram_tensor("yh", [128, t_loc], F16, kind="ExternalOutput")

    with tile.TileContext(nc) as tc:
        with (
            tc.tile_pool(name="sb", bufs=1) as sp,
            tc.tile_pool(name="ps", bufs=PS_BUFS, space="PSUM") as psp,
        ):
            # ---- ALL input supertile DMAs, hoisted to the front of the
            # Sync HWDGE ring. Every supertile gets its own SBUF tile (x is
            # only 33 KB/partition in fp8), so no trigger ever blocks on a
            # buffer slot and the ring streams the whole input back-to-back
            # while the PE chews from the front. ----
            xts = []
            off = 0
            for si, w in enumerate(widths):
                xt = sp.tile([128, w + 2], F8E3, tag=f"xt{si}", bufs=1)
                nc.sync.dma_start(xt, xh[:, off : off + w + 2])
                xts.append(xt)
                off += w
            nzts = []
            if with_noise:
                off = 0
                for si, w in enumerate(widths):
                    nzt = sp.tile([BG, w], F16, tag=f"nzt{si}", bufs=1)
                    nc.sync.dma_start(nzt, nz[:, off : off + w])
                    nzts.append(nzt)
                    off += w

            # ---- constants ride the (otherwise idle at start) Scalar-engine
            # HWDGE ring so they land in parallel with the first supertile ----
            wt = sp.tile([128, K, 128], F16, tag="wt", bufs=1)
            nc.scalar.dma_start(wt, wtk[:, :].rearrange("p (k m) -> p k m", k=K))
            if with_bias:
                bia_sb = sp.tile([128, 1], F32, tag="bia", bufs=1)
                nc.scalar.dma_start(bia_sb, bia[:, :])
            if with_noise:
                wn_sb = sp.tile([BG, 128], F16, tag="wn", bufs=1)
                nc.scalar.dma_start(wn_sb, wnd[:, :])

            # ---- warmup: a few matmuls on scratch so the PE clock ramps
            # while the first x supertile is still in flight (results land
            # in a PSUM slot that a real chunk later overwrites). The memset
            # runs on GpSimd, which reaches kernel-body work ~1.5us before
            # DVE/ACT do, so the warmups start as early as possible. ----
            scr = sp.tile([128, MMN], F16, tag="scr", bufs=1)
            nc.gpsimd.memset(scr, 0.0)
            ps_w = psp.tile([128, CH], F32, tag="ps")
            for _ in range(N_WARM):
                nc.tensor.matmul(
                    ps_w[:, 0:MMN], scr[:, 0:128], scr,
                    start=True, stop=True, skip_group_check=True,
                )

            # ---- main loop: chunks of CH cols through PSUM; one ACT Prelu
            # per piece straight from PSUM; one output DMA per chunk on the
            # Sync ring (program order puts them after all input triggers,
            # so the input stream is never head-of-line blocked). ----
            off = 0
            for si, w in enumerate(widths):
                xt = xts[si]
                is_tail_st = si >= len(widths) - N_TAIL_ST
                for h in range(0, w, CH):
                    cw = min(CH, w - h)
                    ng = cw // MMN
                    ps = psp.tile([128, CH], F32, tag="ps")
                    for k in range(K):
                        for g in range(ng):
                            col = h + g * MMN + k
                            nc.tensor.matmul(
                                ps[:, g * MMN : (g + 1) * MMN],
                                wt[:, k, :],
                                xt[:, col : col + MMN],
                                start=(k == 0),
                                stop=(k == K - 1 and not with_noise),
                                skip_group_check=True,
                            )
                    if with_noise:
                        for g in range(ng):
                            nc.tensor.matmul(
                                ps[:, g * MMN : (g + 1) * MMN],
                                wn_sb[:, :],
                                nzts[si][:, h + g * MMN : h + (g + 1) * MMN],
                                start=False,
                                stop=True,
                                skip_group_check=True,
                            )
                    # epilogue: one ACT op per piece does the whole thing:
                    # fp16 out = Prelu(psum + bias) with alpha=0.2, straight
                    # from PSUM (parametric_relu takes alpha as an operand;
                    # Lrelu would use a baked-alpha table). Tail pieces are
                    # 512 cols so the serial ACT->DMA chain after the final
                    # matmul covers little data.
                    ot = sp.tile([128, cw], F16, tag=f"ot{si}_{h}", bufs=1)
                    psz = 512 if is_tail_st else min(1024, cw)
                    for p in range(0, cw, psz):
                        if use_act_lrelu:
                            nc.scalar.activation(
                                ot[:, p : p + psz], ps[:, p : p + psz],
                                mybir.ActivationFunctionType.Prelu,
                                bias=(bia_sb[:, 0:1] if with_bias else 0.0),
                                scale=1.0, alpha=ALPHA,
                            )
                        else:
                            z = sp.tile([128, cw], F16, tag=f"z{cw}", bufs=3)
                            if with_bias:
                                nc.scalar.activation(
                                    z[:, p : p + psz], ps[:, p : p + psz],
                                    mybir.ActivationFunctionType.Identity,
                                    bias=bia_sb[:, 0:1], scale=1.0,
                                )
                            else:
                                nc.scalar.activation(
                                    z[:, p : p + psz], ps[:, p : p + psz],
                                    mybir.ActivationFunctionType.Copy,
                                    bias=0.0, scale=1.0,
                                )
                            nc.vector.scalar_tensor_tensor(
                                ot[:, p : p + psz], z[:, p : p + psz], ALPHA,
                                z[:, p : p + psz], op0=mult, op1=amax,
                            )
                    # the very last chunk's trigger rides the Scalar ring so
                    # it issues FIFO right behind its own ACT piece instead
                    # of queueing behind other triggers on the Sync ring
                    is_last_chunk = (si == len(widths) - 1) and (h + cw >= w)
                    if is_last_chunk:
                        for p in range(0, cw, psz):
                            nc.scalar.dma_start(
                                yh[:, off + h + p : off + h + p + psz],
                                ot[:, p : p + psz],
                            )
                    elif is_tail_st:
                        for p in range(0, cw, psz):
                            nc.sync.dma_start(
                                yh[:, off + h + p : off + h + p + psz],
                                ot[:, p : p + psz],
                            )
                    else:
                        nc.sync.dma_start(
                            yh[:, off + h : off + h + cw], ot[:, 0:cw]
                        )
                off += w

    nc.compile()
    return nc


def _modulated_taps(style, fc_weight, fc_bias, weight):
    """Host-side style affine + modulate + demodulate, fp32 exact."""
    s = style @ (fc_weight * GAIN).T + fc_bias
    s = np.where(s >= 0, s, ALPHA * s)                        # (B, CIN)
    w = weight[None] * s[:, None, :, None]                    # (B, COUT, CIN, K)
    d = 1.0 / np.sqrt((w * w).sum(axis=(2, 3)) + EPS)         # (B, COUT)
    return w * d[:, :, None, None]


def shard_inputs(x, style, fc_weight, fc_bias, weight, bias, noise_strength,
                 noise, t_loc=T_LOC, force_noise=False, force_bias=False):
    """Build the 8 per-core input dicts.

    Returns (in_maps, with_noise, with_bias)."""
    x = np.asarray(x, dtype=np.float32)
    style = np.asarray(style, dtype=np.float32)
    fc_weight = np.asarray(fc_weight, dtype=np.float32)
    fc_bias = np.asarray(fc_bias, dtype=np.float32)
    weight = np.asarray(weight, dtype=np.float32)
    bias = np.asarray(bias, dtype=np.float32)
    noise_strength = np.asarray(noise_strength, dtype=np.float32)
    noise = np.asarray(noise, dtype=np.float32)

    wf = _modulated_taps(style, fc_weight, fc_bias, weight).astype(np.float16)
    x16 = x.astype(ml_dtypes.float8_e3m4)
    with_noise = bool(np.any(noise_strength != 0)) or force_noise
    with_bias = bool(np.any(bias != 0)) or force_bias

    b_, cin_, t_ = x.shape
    tsplit = t_ // t_loc

    in_maps = []
    for c in range(b_ // BG * tsplit):
        g, h = divmod(c, tsplit)
        wtp = np.zeros((128, K * 128), dtype=np.float16)
        for i in range(BG):
            # block for sample BG*g + i: rows 32i..32i+32 = cin, cols = cout
            for k in range(K):
                wtp[32 * i : 32 * i + 32, 128 * k + 32 * i : 128 * k + 32 * i + 32] = (
                    wf[BG * g + i][:, :, k].T
                )
        xs = x16[BG * g : BG * g + BG]                        # [4, 32, T]
        xpad = np.zeros((BG, cin_, t_loc + 2), dtype=ml_dtypes.float8_e3m4)
        lo = h * t_loc - 1
        hi = h * t_loc + t_loc + 1
        src_lo, src_hi = max(lo, 0), min(hi, t_)
        xpad[:, :, src_lo - lo : src_lo - lo + (src_hi - src_lo)] = (
            xs[:, :, src_lo:src_hi]
        )
        m = {
            "xh": np.ascontiguousarray(xpad.reshape(128, t_loc + 2)),
            "wtk": wtp,
        }
        if with_bias:
            m["bia"] = np.tile(bias, BG).reshape(128, 1).copy()
        if with_noise:
            m["nz"] = np.ascontiguousarray(
                noise[BG * g : BG * g + BG, 0, h * t_loc : (h + 1) * t_loc]
            ).astype(np.float16)
            wn = np.zeros((BG, 128), dtype=np.float16)
            for i in range(BG):
                wn[i, 32 * i : 32 * i + 32] = noise_strength.astype(np.float16)
            m["wnd"] = wn
        in_maps.append(m)
    return in_maps, with_noise, with_bias


def unshard_output(results, b_=B, t_loc=T_LOC, tsplit=TSPLIT):
    y = np.empty((b_, COUT, t_loc * tsplit), dtype=np.float32)
    for c, r in enumerate(results):
        g, h = divmod(c, tsplit)
        y[BG * g : BG * g + BG, :, h * t_loc : (h + 1) * t_loc] = (
            np.asarray(r["yh"]).astype(np.float32).reshape(BG, COUT, t_loc)
        )
    return y


_PROGRAM_CACHE = {}


def get_program(with_noise=False, with_bias=False):
    key = (with_noise, with_bias)
    if key not in _PROGRAM_CACHE:
        _PROGRAM_CACHE[key] = build_program(
            with_noise=with_noise, with_bias=with_bias
        )
    return _PROGRAM_CACHE[key]


def kernel(x, style, fc_weight, fc_bias, weight, bias, noise_strength, noise):
    from concourse import bass_utils

    in_maps, with_noise, with_bias = shard_inputs(
        x, style, fc_weight, fc_bias, weight, bias, noise_strength, noise
    )
    nc = get_program(with_noise=with_noise, with_bias=with_bias)
    res = bass_utils.run_bass_kernel_spmd(nc, in_maps, core_ids=list(range(N_CORES)))
    return unshard_output(res.results)


# revision 17
# speedup vs baseline: 1.0005x; 1.0005x over previous
"""Style-modulated Conv1d (StyleGAN-like) Trainium2 kernel.

Full-input contract: kernel(**inputs) takes the unsharded fp32 inputs and
returns the full (B, COUT, T) fp32 output. Internally the work is sharded
over 8 NeuronCores: batch-groups of 4 samples x T-halves (4x2 grid), so each
core processes a [128, T/2] slab at full partition occupancy.

The style modulation is folded on the host: with
  s = lrelu(style @ (fc_w * gain)^T + fc_b)          (B, CIN)
  d = rsqrt(sum_{cin,k} (w * s)^2 + eps)             (B, COUT)
the modulated-demodulated conv is an ordinary conv with per-sample taps
  w_final[b, cout, cin, k] = w[cout, cin, k] * s[b, cin] * d[b, cout]
followed by  y = lrelu(conv + nstr*noise + bias).  The taps (a few KB) are
built in fp32 on the host, cast to fp16, and packed block-diagonally over the
4 samples of each core's batch group, so the whole device program is just
K=3 shifted [128x128]x[128x512] matmuls accumulating in PSUM plus a single
ACT Lrelu epilogue (PSUM -> SBUF fp16) per chunk.

x moves over HBM in fp8_e3m4 (TRN FP8_EXP3, 4 mantissa bits) and is fed to
the PE directly as the fp8 moving operand against fp16 stationary taps (the
PE upconverts both to fp22 internally, so no cast pass is needed). y moves
in fp16. End-to-end rel err of the fp8-in pipeline vs the fp32 reference is
1.57e-2 (validated bit-exact against a host emulation), inside the 2e-2
gate. The kernel is PE-bound: 3 taps x 32768 cols = 98304 PE cycles ~ 41us;
HBM traffic is 4.2 MB in + 8.4 MB out per core (~35us at 358 GB/s).

Schedule notes (from NTFF traces):
- input DMAs ride the Sync-engine HWDGE ring; body output DMAs ride the
  GpSimd SWDGE ring (its ~3.5us exit drain overlaps the tail); the last two
  supertiles' outputs are deferred to the Sync ring so nothing waits on the
  GpSimd drain;
- a few warmup matmuls on an uninitialized scratch tile start the PE HAM
  clock ramp while the first x supertile is in flight;
- the epilogue is one ACT op per 1024-col piece: Lrelu(psum*1 + bias) with
  alpha=0.2, writing fp16 straight to the output staging tile (DVE is idle);
- all SBUF tiles live in ONE tile pool (per-tag ring buffers) so the
  end-of-program pool-release barrier storm collapses to a single release.
"""

import ml_dtypes
import numpy as np

import concourse.bass as bass
import concourse.tile as tile
from concourse import bacc, mybir

F32 = mybir.dt.float32
F16 = mybir.dt.float16
F8E3 = mybir.dt.float8e3  # TRN FP8_EXP3 (e3m4): 4 mantissa bits

B, CIN, COUT, T, WDIM, K = 16, 32, 32, 65536, 512, 3
ALPHA = 0.2
GAIN = float(1.0 / np.sqrt(np.float32(WDIM)))
EPS = 1e-8

N_CORES = 8
BG = 4          # samples per core (batch group)
TSPLIT = 2      # T split factor
T_LOC = T // TSPLIT

CH = 1024       # compute chunk columns (one 2-bank PSUM tile, per-chunk out DMA)
MMN = 512       # matmul free dim (one PSUM bank of fp32)
PS_BUFS = 4     # PSUM rotation depth (4 x 2 banks = all 8 banks)
N_WARM = 5      # warmup matmuls to start the HAM clock ramp
N_TAIL_ST = 3   # trailing supertiles with 512-col pieces (short serial tail)


def _supertile_schedule(t_loc):
    """Input-DMA granularity: small head tiles (fast first matmul), 4k body,
    small tail (short drain)."""
    head = [512, 1024, 2048]
    tail = [2048, 512, 512, 512]
    body = t_loc - sum(head) - sum(tail)
    assert body >= 0 and body % 1024 == 0
    widths = head + [4096] * (body // 4096)
    if body % 4096:
        widths.append(body % 4096)
    widths += tail
    assert sum(widths) == t_loc
    return widths


def build_program(t_loc=T_LOC, with_noise=False, with_bias=False,
                  use_act_lrelu=True):
    """One-core Bass program; identical on all 8 cores (SPMD, data differs)."""
    widths = _supertile_schedule(t_loc)
    mult = mybir.AluOpType.mult
    amax = mybir.AluOpType.max

    nc = bacc.Bacc("TRN2", target_bir_lowering=False, debug=False)
    xh = nc.dram_tensor("xh", [128, t_loc + 2], F8E3, kind="ExternalInput")
    wtk = nc.dram_tensor("wtk", [128, K * 128], F16, kind="ExternalInput")
    if with_bias:
        bia = nc.dram_tensor("bia", [128, 1], F32, kind="ExternalInput")
    if with_noise:
        nz = nc.dram_tensor("nz", [BG, t_loc], F16, kind="ExternalInput")
        wnd = nc.dram_tensor("wnd", [BG, 128], F16, kind="ExternalInput")
    yh = nc.dram_tensor("yh", [128, t_loc], F16, kind="ExternalOutput")

    with tile.TileContext(nc) as tc:
        with (
            tc.tile_pool(name="sb", bufs=1) as sp,
            tc.tile_pool(name="ps", bufs=PS_BUFS, space="PSUM") as psp,
        ):
            # ---- ALL input supertile DMAs, hoisted to the front of the
            # Sync HWDGE ring. Every supertile gets its own SBUF tile (x is
            # only 33 KB/partition in fp8), so no trigger ever blocks on a
            # buffer slot and the ring streams the whole input back-to-back
            # while the PE chews from the front. ----
            xts = []
            off = 0
            wt = None
            for si, w in enumerate(widths):
                xt = sp.tile([128, w + 2], F8E3, tag=f"xt{si}", bufs=1)
                nc.sync.dma_start(xt, xh[:, off : off + w + 2])
                xts.append(xt)
                off += w
                if si == 0:
                    # taps right behind the first supertile: both must land
                    # before the first real matmul. (NOT on the Scalar ring:
                    # qActDynamicHW maps to the runtime-internal queue row
                    # Q_XIV whose completion latency is erratic.)
                    wt = sp.tile([128, K, 128], F16, tag="wt", bufs=1)
                    nc.sync.dma_start(
                        wt, wtk[:, :].rearrange("p (k m) -> p k m", k=K)
                    )
            nzts = []
            if with_noise:
                off = 0
                for si, w in enumerate(widths):
                    nzt = sp.tile([BG, w], F16, tag=f"nzt{si}", bufs=1)
                    nc.sync.dma_start(nzt, nz[:, off : off + w])
                    nzts.append(nzt)
                    off += w

            # ---- remaining constants (tiny, once) ----
            if with_bias:
                bia_sb = sp.tile([128, 1], F32, tag="bia", bufs=1)
                nc.sync.dma_start(bia_sb, bia[:, :])
            if with_noise:
                wn_sb = sp.tile([BG, 128], F16, tag="wn", bufs=1)
                nc.sync.dma_start(wn_sb, wnd[:, :])

            # ---- warmup: a few matmuls on scratch so the PE clock ramps
            # while the first x supertile is still in flight (results land
            # in a PSUM slot that a real chunk later overwrites). The memset
            # runs on GpSimd, which reaches kernel-body work ~1.5us before
            # DVE/ACT do, so the warmups start as early as possible. ----
            scr = sp.tile([128, MMN], F16, tag="scr", bufs=1)
            nc.gpsimd.memset(scr, 0.0)
            ps_w = psp.tile([128, CH], F32, tag="ps")
            for _ in range(N_WARM):
                nc.tensor.matmul(
                    ps_w[:, 0:MMN], scr[:, 0:128], scr,
                    start=True, stop=True, skip_group_check=True,
                )

            # ---- main loop: chunks of CH cols through PSUM; one ACT Prelu
            # per piece straight from PSUM; one output DMA per chunk on the
            # Sync ring (program order puts them after all input triggers,
            # so the input stream is never head-of-line blocked). ----
            off = 0
            for si, w in enumerate(widths):
                xt = xts[si]
                is_tail_st = si >= len(widths) - N_TAIL_ST
                for h in range(0, w, CH):
                    cw = min(CH, w - h)
                    ng = cw // MMN
                    ps = psp.tile([128, CH], F32, tag="ps")
                    for k in range(K):
                        for g in range(ng):
                            col = h + g * MMN + k
                            nc.tensor.matmul(
                                ps[:, g * MMN : (g + 1) * MMN],
                                wt[:, k, :],
                                xt[:, col : col + MMN],
                                start=(k == 0),
                                stop=(k == K - 1 and not with_noise),
                                skip_group_check=True,
                            )
                    if with_noise:
                        for g in range(ng):
                            nc.tensor.matmul(
                                ps[:, g * MMN : (g + 1) * MMN],
                                wn_sb[:, :],
                                nzts[si][:, h + g * MMN : h + (g + 1) * MMN],
                                start=False,
                                stop=True,
                                skip_group_check=True,
                            )
                    # epilogue: one ACT op per piece does the whole thing:
                    # fp16 out = Prelu(psum + bias) with alpha=0.2, straight
                    # from PSUM (parametric_relu takes alpha as an operand;
                    # Lrelu would use a baked-alpha table). Tail pieces are
                    # 512 cols so the serial ACT->DMA chain after the final
                    # matmul covers little data.
                    ot = sp.tile([128, cw], F16, tag=f"ot{si}_{h}", bufs=1)
                    psz = 512 if is_tail_st else min(1024, cw)
                    for p in range(0, cw, psz):
                        if use_act_lrelu:
                            nc.scalar.activation(
                                ot[:, p : p + psz], ps[:, p : p + psz],
                                mybir.ActivationFunctionType.Prelu,
                                bias=(bia_sb[:, 0:1] if with_bias else 0.0),
                                scale=1.0, alpha=ALPHA,
                            )
                        else:
                            z = sp.tile([128, cw], F16, tag=f"z{cw}", bufs=3)
                            if with_bias:
                                nc.scalar.activation(
                                    z[:, p : p + psz], ps[:, p : p + psz],
                                    mybir.ActivationFunctionType.Identity,
                                    bias=bia_sb[:, 0:1], scale=1.0,
                                )
                            else:
                                nc.scalar.activation(
                                    z[:, p : p + psz], ps[:, p : p + psz],
                                    mybir.ActivationFunctionType.Copy,
                                    bias=0.0, scale=1.0,
                                )
                            nc.vector.scalar_tensor_tensor(
                                ot[:, p : p + psz], z[:, p : p + psz], ALPHA,
                                z[:, p : p + psz], op0=mult, op1=amax,
                            )
                    if is_tail_st:
                        for p in range(0, cw, psz):
                            nc.sync.dma_start(
                                yh[:, off + h + p : off + h + p + psz],
                                ot[:, p : p + psz],
                            )
                    else:
                        nc.sync.dma_start(
                            yh[:, off + h : off + h + cw], ot[:, 0:cw]
                        )
                off += w

    nc.compile()
    return nc


def _modulated_taps(style, fc_weight, fc_bias, weight):
    """Host-side style affine + modulate + demodulate, fp32 exact."""
    s = style @ (fc_weight * GAIN).T + fc_bias
    s = np.where(s >= 0, s, ALPHA * s)                        # (B, CIN)
    w = weight[None] * s[:, None, :, None]                    # (B, COUT, CIN, K)
    d = 1.0 / np.sqrt((w * w).sum(axis=(2, 3)) + EPS)         # (B, COUT)
    return w * d[:, :, None, None]


def shard_inputs(x, style, fc_weight, fc_bias, weight, bias, noise_strength,
                 noise, t_loc=T_LOC, force_noise=False, force_bias=False):
    """Build the 8 per-core input dicts.

    Returns (in_maps, with_noise, with_bias)."""
    x = np.asarray(x, dtype=np.float32)
    style = np.asarray(style, dtype=np.float32)
    fc_weight = np.asarray(fc_weight, dtype=np.float32)
    fc_bias = np.asarray(fc_bias, dtype=np.float32)
    weight = np.asarray(weight, dtype=np.float32)
    bias = np.asarray(bias, dtype=np.float32)
    noise_strength = np.asarray(noise_strength, dtype=np.float32)
    noise = np.asarray(noise, dtype=np.float32)

    wf = _modulated_taps(style, fc_weight, fc_bias, weight).astype(np.float16)
    x16 = x.astype(ml_dtypes.float8_e3m4)
    with_noise = bool(np.any(noise_strength != 0)) or force_noise
    with_bias = bool(np.any(bias != 0)) or force_bias

    b_, cin_, t_ = x.shape
    tsplit = t_ // t_loc

    in_maps = []
    for c in range(b_ // BG * tsplit):
        g, h = divmod(c, tsplit)
        wtp = np.zeros((128, K * 128), dtype=np.float16)
        for i in range(BG):
            # block for sample BG*g + i: rows 32i..32i+32 = cin, cols = cout
            for k in range(K):
                wtp[32 * i : 32 * i + 32, 128 * k + 32 * i : 128 * k + 32 * i + 32] = (
                    wf[BG * g + i][:, :, k].T
                )
        xs = x16[BG * g : BG * g + BG]                        # [4, 32, T]
        xpad = np.zeros((BG, cin_, t_loc + 2), dtype=ml_dtypes.float8_e3m4)
        lo = h * t_loc - 1
        hi = h * t_loc + t_loc + 1
        src_lo, src_hi = max(lo, 0), min(hi, t_)
        xpad[:, :, src_lo - lo : src_lo - lo + (src_hi - src_lo)] = (
            xs[:, :, src_lo:src_hi]
        )
        m = {
            "xh": np.ascontiguousarray(xpad.reshape(128, t_loc + 2)),
            "wtk": wtp,
        }
        if with_bias:
            m["bia"] = np.tile(bias, BG).reshape(128, 1).copy()
        if with_noise:
            m["nz"] = np.ascontiguousarray(
                noise[BG * g : BG * g + BG, 0, h * t_loc : (h + 1) * t_loc]
            ).astype(np.float16)
            wn = np.zeros((BG, 128), dtype=np.float16)
            for i in range(BG):
                wn[i, 32 * i : 32 * i + 32] = noise_strength.astype(np.float16)
            m["wnd"] = wn
        in_maps.append(m)
    return in_maps, with_noise, with_bias


def unshard_output(results, b_=B, t_loc=T_LOC, tsplit=TSPLIT):
    y = np.empty((b_, COUT, t_loc * tsplit), dtype=np.float32)
    for c, r in enumerate(results):
        g, h = divmod(c, tsplit)
        y[BG * g : BG * g + BG, :, h * t_loc : (h + 1) * t_loc] = (
            np.asarray(r["yh"]).astype(np.float32).reshape(BG, COUT, t_loc)
        )
    return y


_PROGRAM_CACHE = {}


def get_program(with_noise=False, with_bias=False):
    key = (with_noise, with_bias)
    if key not in _PROGRAM_CACHE:
        _PROGRAM_CACHE[key] = build_program(
            with_noise=with_noise, with_bias=with_bias
        )
    return _PROGRAM_CACHE[key]


def kernel(x, style, fc_weight, fc_bias, weight, bias, noise_strength, noise):
    from concourse import bass_utils

    in_maps, with_noise, with_bias = shard_inputs(
        x, style, fc_weight, fc_bias, weight, bias, noise_strength, noise
    )
    nc = get_program(with_noise=with_noise, with_bias=with_bias)
    res = bass_utils.run_bass_kernel_spmd(nc, in_maps, core_ids=list(range(N_CORES)))
    return unshard_output(res.results)
